# revision 1
# baseline (speedup 1.0000x reference)
"""Trainium2 Bass kernel for nn_CA1AttentionGate.

Computes, for full inputs (B=1, S=8192, H=1024, F=128, K=2):
    temporal = relu(t @ Wt1 + bt1) @ Wt2 + bt2          [K,F]
    mem      = dg_features + temporal                    [K,F]
    qmean    = query.mean(axis=1)                        [1,H]
    score_k  = tanh([mem_k ; qmean] @ Wa1 + ba1) @ Wa2 + ba2
    w_k      = sigmoid(score_k)
    g_k      = mem_k @ Wg + bg                           [K,H]
    row[s]   = (1/K) * sum_k w_k * (g_k . key[s])        [S]
    out      = broadcast(row) -> [1,1,S,S]

Sharding: sequence-parallel over the key/seq axis across 8 cores.  Each
core computes the final gate row for its 1024 key positions and writes
its dense [8192, 1024] column slab of the output.  The only cross-core
quantity is qmean: each core reduces its query shard and a 4KB AllReduce
completes the mean (fallback variant replicates the full query read).
"""

import os

import numpy as np

SEQ = 8192
H = 1024
F = 128
K = 2
NCORES = 8
SHARD = SEQ // NCORES  # 1024
NT = SHARD // 128  # 8 key tiles per shard

_PROG_CACHE = {}


def _build(use_collective: bool):
    import concourse.bacc as bacc
    import concourse.bass as bass
    import concourse.tile as tile
    from concourse import mybir
    from concourse.tile_rust import add_dep_helper

    AF = mybir.ActivationFunctionType
    ALU = mybir.AluOpType
    f32 = mybir.dt.float32

    nc = bacc.Bacc(
        "TRN2",
        target_bir_lowering=False,
        debug=False,
        num_devices=NCORES,
    )

    def din(name, shape):
        return nc.dram_tensor(name, list(shape), f32, kind="ExternalInput").ap()

    q_rows = SHARD if use_collective else SEQ
    qs = din("qs", (q_rows, H))
    ks = din("ks", (SHARD, H))
    dg = din("dg", (K, F))
    ts = din("ts", (K,))
    Wt1 = din("Wt1", (1, F // 4))
    bt1 = din("bt1", (F // 4,))
    Wt2 = din("Wt2", (F // 4, F))
    bt2 = din("bt2", (F,))
    Wa1 = din("Wa1", (F + H, F))
    ba1 = din("ba1", (F,))
    Wa2 = din("Wa2", (F, 1))
    ba2 = din("ba2", (1,))
    Wg = din("Wg", (F, H))
    bg = din("bg", (H,))
    # column of 1/SEQ: the qsum partition-reduce matmul yields the scaled
    # mean contribution directly
    scale_col = din("scale_col", (128, 1))
    out = nc.dram_tensor("out", [SEQ, SHARD], f32, kind="ExternalOutput").ap()

    def bcast(ap, n):
        # replicate a DRAM row across n partitions (stride-0 partition dim)
        return bass.AP(tensor=ap.tensor, offset=ap.offset, ap=[[0, n]] + list(ap.ap))

    def col(ap, n):
        # load a flat [n] DRAM vector as an [n, 1] column
        return bass.AP(tensor=ap.tensor, offset=ap.offset, ap=[[1, n], [n, 1]])

    with tile.TileContext(nc) as tc:
        with (
            tc.tile_pool(name="consts", bufs=1) as cp,
            tc.tile_pool(name="work", bufs=1) as wp,
            tc.tile_pool(name="qstream", bufs=8) as qp,
            tc.tile_pool(name="scratch", bufs=3) as sp,
            tc.tile_pool(name="psum_small", bufs=2, space="PSUM") as pps,
            tc.tile_pool(name="psum_big", bufs=3, space="PSUM") as ppb,
            tc.tile_pool(name="dram", bufs=1, space="DRAM") as dp,
        ):
            # ---- constant / weight loads (sync DGE ring) ---------------
            sc_c = cp.tile([128, 1], f32)
            nc.sync.dma_start(sc_c, scale_col)
            # ---- query shard DMAs get the wire first -------------------
            # (every weight/const below has >=10us of slack; the query
            # stream feeds the collective and must not queue behind them)
            nq = q_rows // 128
            qv = qs.rearrange("(t p) h -> t p h", p=128)
            qtiles = []
            q_insts = []
            for i in range(nq):
                qt = qp.tile([128, H], f32, tag="qt")
                q_insts.append(nc.sync.dma_start(qt, qv[i]))
                qtiles.append(qt)

            Wt2_sb = cp.tile([F // 4, F], f32)
            nc.sync.dma_start(Wt2_sb, Wt2)
            Wa1m_sb = cp.tile([128, 128], f32)
            nc.sync.dma_start(Wa1m_sb, Wa1[0:F, :])
            # qmean rows of Wa1 re-paired to the interleaved qmT layout:
            # chunk c pairs with rows {128 + i*8 + c}
            Wa1q_sb = cp.tile([128, 8, 128], f32)
            nc.sync.dma_start(
                Wa1q_sb, Wa1[F : F + H, :].rearrange("(i c) f -> i c f", c=8)
            )
            Wa2_sb = cp.tile([F, 1], f32)
            nc.sync.dma_start(Wa2_sb, Wa2)
            Wg_sb = cp.tile([F, H], f32)
            nc.sync.dma_start(Wg_sb, Wg)
            dgT_sb = cp.tile([F, K], f32)
            nc.sync.dma_start(dgT_sb, dg.rearrange("k f -> f k"))
            tb_sb = cp.tile([F // 4, K], f32)
            nc.sync.dma_start(tb_sb, bcast(ts, F // 4))
            Wt1T_sb = cp.tile([F // 4, 1], f32)
            nc.sync.dma_start(Wt1T_sb, col(Wt1, F // 4))
            bt1T_sb = cp.tile([F // 4, 1], f32)
            nc.sync.dma_start(bt1T_sb, col(bt1, F // 4))
            bt2T_sb = cp.tile([F, 1], f32)
            nc.sync.dma_start(bt2T_sb, col(bt2, F))
            ba1T_sb = cp.tile([F, 1], f32)
            nc.sync.dma_start(ba1T_sb, col(ba1, F))
            ba2b_sb = cp.tile([1, 1], f32)
            nc.sync.dma_start(ba2b_sb, bcast(ba2, 1))
            bg_sb = cp.tile([1, H], f32)
            nc.sync.dma_start(bg_sb, bg.rearrange("(a h) -> a h", a=1))

            # warm the ACT function tables used late in the critical path
            warm1 = cp.tile([1, 1], f32)
            nc.scalar.activation(warm1, sc_c[0:1, :], AF.Tanh)
            warm2 = cp.tile([1, 1], f32)
            nc.scalar.activation(warm2, sc_c[0:1, :], AF.Sigmoid)
            # key shard: interleaved, ktiles[j][p, :] = ks[p*NT + j, :];
            # explicitly ordered after the query stream so the query mean
            # (-> collective) is not starved of read bandwidth
            kv = ks.rearrange("(p t) h -> p t h", t=NT)
            ktiles = []
            for j in range(NT):
                kt = cp.tile([128, H], f32, tag=f"ks{j}")
                ki = nc.sync.dma_start(kt, kv[:, j, :])
                add_dep_helper(ki.ins, q_insts[-1].ins,
                               reason="key reads after query")
                ktiles.append(kt)

            # ---- query accumulate on DVE: head of the critical path ----
            qacc = wp.tile([128, H], f32)
            for i in range(nq):
                if i == 0:
                    nc.vector.tensor_copy(qacc, qtiles[i])
                else:
                    nc.vector.tensor_add(qacc, qacc, qtiles[i])

            # ---- qmean partial (PE first) -> collective ----------------
            qsum_ps = ppb.tile([1, H], f32, tag="big")
            nc.tensor.matmul(
                qsum_ps[:, 0:512], lhsT=sc_c, rhs=qacc[:, 0:512],
                start=True, stop=True,
            )
            nc.tensor.matmul(
                qsum_ps[:, 512:1024], lhsT=sc_c, rhs=qacc[:, 512:1024],
                start=True, stop=True,
            )
            qpart_sb = wp.tile([1, H], f32)
            nc.scalar.copy(qpart_sb, qsum_ps)
            if use_collective:
                cc_in = dp.tile([1, H], f32)
                cc_out = dp.tile([NCORES, H], f32)
                nc.scalar.dma_start(cc_in, qpart_sb)
                nc.gpsimd.collective_compute(
                    "AllGather",
                    ALU.bypass,
                    replica_groups=[list(range(NCORES))],
                    ins=[cc_in.opt()],
                    outs=[cc_out.opt()],
                )
                # park the gather-result load on the (idle) sync ring
                qmTd8 = wp.tile([128, NCORES, 8], f32)
                nc.sync.dma_start(
                    qmTd8, cc_out[:, :].rearrange("d (p c) -> p d c", c=8)
                )

            # ---- temporal MLP -> memT [F, K] ---------------------------
            h1T = wp.tile([F // 4, K], f32)
            nc.vector.tensor_scalar_mul(h1T, tb_sb, Wt1T_sb)
            nc.vector.tensor_scalar_add(h1T, h1T, bt1T_sb)
            nc.vector.tensor_relu(h1T, h1T)
            tT_ps = pps.tile([F, K], f32, tag="small")
            nc.tensor.matmul(tT_ps, lhsT=Wt2_sb, rhs=h1T, start=True, stop=True)
            memT_sb = wp.tile([F, K], f32)
            nc.scalar.activation(memT_sb, tT_ps, AF.Identity, bias=bt2T_sb, scale=1.0)
            nc.vector.tensor_add(memT_sb, memT_sb, dgT_sb)

            # ---- gate rows g_k = mem_k @ Wg + bg  [1, H] ---------------
            def g_row(k):
                g_ps = ppb.tile([1, H], f32, tag="big")
                nc.tensor.matmul(g_ps[:, 0:512], lhsT=memT_sb[:, k : k + 1],
                                 rhs=Wg_sb[:, 0:512], start=True, stop=True)
                nc.tensor.matmul(g_ps[:, 512:1024], lhsT=memT_sb[:, k : k + 1],
                                 rhs=Wg_sb[:, 512:1024], start=True, stop=True)
                return g_ps

            g0_ps = g_row(0)
            g0_sb = wp.tile([1, H], f32, tag="g0r")
            nc.vector.tensor_add(g0_sb, g0_ps, bg_sb)
            gb0 = wp.tile([128, H], f32, tag="gb0")
            nc.gpsimd.partition_broadcast(gb0[:, :], g0_sb[:, :])
            g1_ps = g_row(1)

            # ---- matvec: DVE muls, ACT accumulate-reductions -----------
            # rcc[p, j, k] = sum_h g_k[h] * ks[p*NT+j, h]
            rcc = wp.tile([128, NT, K], f32)

            def matvec(k, gb, js):
                for j in js:
                    prod = sp.tile([128, H], f32, tag="prod")
                    nc.vector.tensor_mul(prod, ktiles[j], gb)
                    junk = sp.tile([128, H], f32, tag="junk")
                    nc.scalar.activation(
                        junk, prod, AF.Copy,
                        accum_out=rcc[:, j, k : k + 1],
                    )

            matvec(0, gb0, range(4))

            # finish g1 mid-stream (its inputs are ready by now)
            g1_sb = wp.tile([1, H], f32, tag="g1r")
            nc.vector.tensor_add(g1_sb, g1_ps, bg_sb)
            gb1 = wp.tile([128, H], f32, tag="gb1")
            nc.gpsimd.partition_broadcast(gb1[:, :], g1_sb[:, :])

            matvec(0, gb0, range(4, NT))
            matvec(1, gb1, range(NT))

            # reshape both anchors at once to an interleaved row:
            # rTi[0, 2*s + k] = r_k[s]   (s = p*NT + j)
            rTi = wp.tile([1, K * SHARD], f32)
            nc.sync.dma_start(rTi[:, :], rcc[:, :, :])

            # ---- post-collective: qmT, scorer, weights -----------------
            # qmT[p, c] = qmean[p*8 + c]  (interleaved reshape layout)
            qmT = wp.tile([128, 8], f32)
            if use_collective:
                # sum gathered partials over d ([p, c, d] view, reduce X)
                nc.vector.tensor_reduce(
                    qmT, qmTd8[:, :, :].rearrange("p d c -> p c d"),
                    axis=mybir.AxisListType.X, op=ALU.add,
                )
            else:
                nc.scalar.dma_start(qmT, qpart_sb[:, :])
            qmTd = wp.tile([128, 8, K], f32)
            nc.vector.tensor_copy(qmTd[:, :, 0:1], qmT[:, :].rearrange("p c -> p c ()"))
            nc.vector.tensor_copy(qmTd[:, :, 1:2], qmT[:, :].rearrange("p c -> p c ()"))
            haT_ps = pps.tile([F, K], f32, tag="small")
            nc.tensor.matmul(haT_ps, lhsT=Wa1m_sb, rhs=memT_sb,
                             start=True, stop=False)
            for c in range(8):
                nc.tensor.matmul(haT_ps, lhsT=Wa1q_sb[:, c, :],
                                 rhs=qmTd[:, c, :], start=False, stop=(c == 7))
            aT_sb = wp.tile([F, K], f32)
            nc.scalar.activation(aT_sb, haT_ps, AF.Tanh, bias=ba1T_sb, scale=1.0)
            scoreT_ps = pps.tile([1, K], f32, tag="small")
            nc.tensor.matmul(scoreT_ps, lhsT=Wa2_sb, rhs=aT_sb, start=True, stop=True)
            wvT_sb = wp.tile([1, K], f32)
            nc.scalar.activation(wvT_sb, scoreT_ps, AF.Sigmoid, bias=ba2b_sb, scale=1.0)
            nc.scalar.mul(wvT_sb, wvT_sb, 1.0 / K)

            # ---- combine anchors in row space, then one broadcast ------
            rt = rTi[:, :]
            r_ev = bass.AP(tensor=rt.tensor, offset=rt.offset,
                           ap=[[K * SHARD, 1], [K, SHARD]])
            r_od = bass.AP(tensor=rt.tensor, offset=rt.offset + 1,
                           ap=[[K * SHARD, 1], [K, SHARD]])
            o_row = wp.tile([1, SHARD], f32)
            o_tmp = wp.tile([1, SHARD], f32)
            nc.vector.tensor_scalar_mul(o_row, r_ev, wvT_sb[0:1, 0:1])
            nc.vector.tensor_scalar_mul(o_tmp, r_od, wvT_sb[0:1, 1:2])
            nc.vector.tensor_add(o_row, o_row, o_tmp)
            out_sb = wp.tile([128, SHARD], f32)
            nc.gpsimd.partition_broadcast(out_sb[:, :], o_row[:, :])

            # ---- output: 64 x [128 rows, SHARD cols], all rows = row ---
            outv = out.rearrange("(b p) n -> b p n", p=128)
            for b in range(SEQ // 128):
                nc.sync.dma_start(outv[b], out_sb)

    nc.compile()
    return nc


def _get_prog(use_collective: bool):
    key = bool(use_collective)
    if key not in _PROG_CACHE:
        _PROG_CACHE[key] = _build(key)
    return _PROG_CACHE[key]


def _make_in_maps(inputs, use_collective: bool):
    q = np.ascontiguousarray(np.asarray(inputs["query"], np.float32)[0])  # [S,H]
    k = np.ascontiguousarray(np.asarray(inputs["key"], np.float32)[0])  # [S,H]
    common = {
        "dg": np.ascontiguousarray(np.asarray(inputs["dg_features"], np.float32)),
        "ts": np.ascontiguousarray(np.asarray(inputs["timestamps"], np.float32)),
        "Wt1": np.ascontiguousarray(np.asarray(inputs["Wt1"], np.float32)),
        "bt1": np.ascontiguousarray(np.asarray(inputs["bt1"], np.float32)),
        "Wt2": np.ascontiguousarray(np.asarray(inputs["Wt2"], np.float32)),
        "bt2": np.ascontiguousarray(np.asarray(inputs["bt2"], np.float32)),
        "Wa1": np.ascontiguousarray(np.asarray(inputs["Wa1"], np.float32)),
        "ba1": np.ascontiguousarray(np.asarray(inputs["ba1"], np.float32)),
        "Wa2": np.ascontiguousarray(np.asarray(inputs["Wa2"], np.float32)),
        "ba2": np.ascontiguousarray(np.asarray(inputs["ba2"], np.float32)),
        "Wg": np.ascontiguousarray(np.asarray(inputs["Wg"], np.float32)),
        "bg": np.ascontiguousarray(np.asarray(inputs["bg"], np.float32)),
        "scale_col": np.full((128, 1), 1.0 / 8192.0, np.float32),
    }
    in_maps = []
    for d in range(NCORES):
        m = dict(common)
        m["ks"] = np.ascontiguousarray(k[d * SHARD : (d + 1) * SHARD])
        if use_collective:
            m["qs"] = np.ascontiguousarray(q[d * SHARD : (d + 1) * SHARD])
        else:
            m["qs"] = q
        in_maps.append(m)
    return in_maps


def _run(inputs, use_collective: bool, trace: bool = False):
    from concourse.bass_utils import run_bass_kernel_spmd

    nc = _get_prog(use_collective)
    in_maps = _make_in_maps(inputs, use_collective)
    res = run_bass_kernel_spmd(
        nc, in_maps, core_ids=list(range(NCORES)), trace=trace
    )
    full = np.empty((1, 1, SEQ, SEQ), np.float32)
    for d in range(NCORES):
        full[0, 0, :, d * SHARD : (d + 1) * SHARD] = res.results[d]["out"]
    return full, res


def kernel(**inputs) -> np.ndarray:
    use_collective = os.environ.get("CA1_NO_COLLECTIVE", "0") != "1"
    try:
        full, _ = _run(inputs, use_collective)
        return full
    except Exception:
        if not use_collective:
            raise
        # fall back to the zero-communication variant (replicated query)
        _PROG_CACHE.pop(True, None)
        full, _ = _run(inputs, False)
        return full



# revision 7
# speedup vs baseline: 1.1705x; 1.1705x over previous
"""Trainium2 Bass kernel for nn_CA1AttentionGate.

Computes, for full inputs (B=1, S=8192, H=1024, F=128, K=2):
    temporal = relu(t @ Wt1 + bt1) @ Wt2 + bt2          [K,F]
    mem      = dg_features + temporal                    [K,F]
    qmean    = query.mean(axis=1)                        [1,H]
    score_k  = tanh([mem_k ; qmean] @ Wa1 + ba1) @ Wa2 + ba2
    w_k      = sigmoid(score_k)
    g_k      = mem_k @ Wg + bg                           [K,H]
    row[s]   = (1/K) * sum_k w_k * (g_k . key[s])        [S]
    out      = broadcast(row) -> [1,1,S,S]

Sharding: sequence-parallel over the key/seq axis across 8 cores.  Each
core computes the final gate row for its 1024 key positions and writes
its dense [8192, 1024] column slab of the output.  The slab is written
in fp16 (well within the 2e-2 tolerance; the host upcasts on gather),
halving the dominant output-write traffic.  The only cross-core
quantity is qmean: each core reduces its query shard via PE matmuls
into PSUM and a 4KB AllGather completes the mean (fallback variant
replicates the full query read instead).
"""

import os

import numpy as np

SEQ = 8192
H = 1024
F = 128
K = 2
NCORES = 8
SHARD = SEQ // NCORES  # 1024
NT = SHARD // 128  # 8 key tiles per shard

_PROG_CACHE = {}

# bisect toggles (debug only; default = optimized path)
_OUT_F32 = os.environ.get("CA1_OUT_F32", "0") == "1"
_QSUM_DVE = os.environ.get("CA1_QSUM_DVE", "0") == "1"
_MATVEC_ACT = os.environ.get("CA1_MATVEC_ACT", "0") == "1"


def _build(use_collective: bool):
    import concourse.bacc as bacc
    import concourse.bass as bass
    import concourse.tile as tile
    from concourse import mybir
    from concourse.tile_rust import add_dep_helper

    AF = mybir.ActivationFunctionType
    ALU = mybir.AluOpType
    f32 = mybir.dt.float32
    f16 = mybir.dt.float16

    nc = bacc.Bacc(
        "TRN2",
        target_bir_lowering=False,
        debug=False,
        num_devices=NCORES,
    )

    def din(name, shape):
        return nc.dram_tensor(name, list(shape), f32, kind="ExternalInput").ap()

    q_rows = SHARD if use_collective else SEQ
    qs = din("qs", (q_rows, H))
    ks = din("ks", (SHARD, H))
    dg = din("dg", (K, F))
    ts = din("ts", (K,))
    Wt1 = din("Wt1", (1, F // 4))
    bt1 = din("bt1", (F // 4,))
    Wt2 = din("Wt2", (F // 4, F))
    bt2 = din("bt2", (F,))
    Wa1 = din("Wa1", (F + H, F))
    ba1 = din("ba1", (F,))
    Wa2 = din("Wa2", (F, 1))
    ba2 = din("ba2", (1,))
    Wg = din("Wg", (F, H))
    bg = din("bg", (H,))
    # column of 1/SEQ: the qsum partition-reduce matmul yields the scaled
    # mean contribution directly
    scale_col = din("scale_col", (128, 1))
    f_out = f32 if _OUT_F32 else f16
    out = nc.dram_tensor("out", [SEQ, SHARD], f_out, kind="ExternalOutput").ap()

    def bcast(ap, n):
        # replicate a DRAM row across n partitions (stride-0 partition dim)
        return bass.AP(tensor=ap.tensor, offset=ap.offset, ap=[[0, n]] + list(ap.ap))

    def col(ap, n):
        # load a flat [n] DRAM vector as an [n, 1] column
        return bass.AP(tensor=ap.tensor, offset=ap.offset, ap=[[1, n], [n, 1]])

    with tile.TileContext(nc) as tc:
        with (
            tc.tile_pool(name="consts", bufs=1) as cp,
            tc.tile_pool(name="work", bufs=1) as wp,
            tc.tile_pool(name="qstream", bufs=8) as qp,
            tc.tile_pool(name="scratch", bufs=2) as sp,
            tc.tile_pool(name="psum_small", bufs=2, space="PSUM") as pps,
            tc.tile_pool(name="psum_big", bufs=3, space="PSUM") as ppb,
            tc.tile_pool(name="dram", bufs=1, space="DRAM") as dp,
        ):
            # ---- scale column first (feeds the qsum matmuls) ------------
            sc_c = cp.tile([128, 1], f32)
            nc.sync.dma_start(sc_c, scale_col)
            # ---- query shard DMAs get the wire next --------------------
            nq = q_rows // 128
            qv = qs.rearrange("(t p) h -> t p h", p=128)
            qtiles = []
            q_insts = []
            for i in range(nq):
                qt = qp.tile([128, H], f32, tag="qt")
                q_insts.append(nc.sync.dma_start(qt, qv[i]))
                qtiles.append(qt)

            # ---- weights needed before the matvec (small) --------------
            Wt2_sb = cp.tile([F // 4, F], f32)
            w_a0 = nc.sync.dma_start(Wt2_sb, Wt2)
            add_dep_helper(w_a0.ins, q_insts[-1].ins,
                           reason="weight reads after query")
            dgT_sb = cp.tile([F, K], f32)
            nc.sync.dma_start(dgT_sb, dg.rearrange("k f -> f k"))
            tb_sb = cp.tile([F // 4, K], f32)
            nc.sync.dma_start(tb_sb, bcast(ts, F // 4))
            Wt1T_sb = cp.tile([F // 4, 1], f32)
            nc.sync.dma_start(Wt1T_sb, col(Wt1, F // 4))
            bt1T_sb = cp.tile([F // 4, 1], f32)
            nc.sync.dma_start(bt1T_sb, col(bt1, F // 4))
            bt2T_sb = cp.tile([F, 1], f32)
            nc.sync.dma_start(bt2T_sb, col(bt2, F))
            Wg_sb = cp.tile([F, H], f32)
            nc.sync.dma_start(Wg_sb, Wg)
            bg_sb = cp.tile([1, H], f32)
            w_last = nc.sync.dma_start(bg_sb, bg.rearrange("(a h) -> a h", a=1))

            # warm the ACT function tables used late in the critical path
            warm1 = cp.tile([1, 1], f32)
            nc.scalar.activation(warm1, sc_c[0:1, :], AF.Tanh)
            warm2 = cp.tile([1, 1], f32)
            nc.scalar.activation(warm2, sc_c[0:1, :], AF.Sigmoid)

            # key shard: interleaved, ktiles[j][p, :] = ks[p*NT + j, :];
            # explicitly ordered after the small-weight block
            kv = ks.rearrange("(p t) h -> p t h", t=NT)
            ktiles = []
            k_insts = []
            for j in range(NT):
                kt = cp.tile([128, H], f32, tag=f"ks{j}")
                ki = nc.sync.dma_start(kt, kv[:, j, :])
                add_dep_helper(ki.ins, w_last.ins,
                               reason="key reads after early weights")
                ktiles.append(kt)
                k_insts.append(ki)

            # ---- scorer weights (needed only post-collective) ----------
            Wa1m_sb = cp.tile([128, 128], f32)
            wb0 = nc.sync.dma_start(Wa1m_sb, Wa1[0:F, :])
            add_dep_helper(wb0.ins, k_insts[-1].ins,
                           reason="scorer weights after key stream")
            # qmean rows of Wa1 re-paired to the interleaved qmT layout:
            # chunk c pairs with rows {128 + i*8 + c}
            Wa1q_sb = cp.tile([128, 8, 128], f32)
            nc.sync.dma_start(
                Wa1q_sb, Wa1[F : F + H, :].rearrange("(i c) f -> i c f", c=8)
            )
            Wa2_sb = cp.tile([F, 1], f32)
            nc.sync.dma_start(Wa2_sb, Wa2)
            ba1T_sb = cp.tile([F, 1], f32)
            nc.sync.dma_start(ba1T_sb, col(ba1, F))
            ba2b_sb = cp.tile([1, 1], f32)
            nc.sync.dma_start(ba2b_sb, bcast(ba2, 1))

            # ---- qsum on PE: psum[0, h] = sum_s q[s, h] / SEQ -----------
            # (accumulating matmuls keep DVE free for the matvec)
            qsum_ps = ppb.tile([1, H], f32, tag="big")
            if _QSUM_DVE:
                qacc = wp.tile([128, H], f32)
                for i in range(nq):
                    if i == 0:
                        nc.vector.tensor_copy(qacc, qtiles[i])
                    else:
                        nc.vector.tensor_add(qacc, qacc, qtiles[i])
                nc.tensor.matmul(qsum_ps[:, 0:512], lhsT=sc_c,
                                 rhs=qacc[:, 0:512], start=True, stop=True)
                nc.tensor.matmul(qsum_ps[:, 512:1024], lhsT=sc_c,
                                 rhs=qacc[:, 512:1024], start=True, stop=True)
            else:
                for i in range(nq):
                    nc.tensor.matmul(
                        qsum_ps[:, 0:512], lhsT=sc_c, rhs=qtiles[i][:, 0:512],
                        start=(i == 0), stop=(i == nq - 1),
                    )
                    nc.tensor.matmul(
                        qsum_ps[:, 512:1024], lhsT=sc_c, rhs=qtiles[i][:, 512:1024],
                        start=(i == 0), stop=(i == nq - 1),
                    )
            qpart_sb = wp.tile([1, H], f32)
            nc.scalar.copy(qpart_sb, qsum_ps)
            if use_collective:
                cc_in = dp.tile([1, H], f32)
                cc_out = dp.tile([NCORES, H], f32)
                nc.scalar.dma_start(cc_in, qpart_sb)
                nc.gpsimd.collective_compute(
                    "AllGather",
                    ALU.bypass,
                    replica_groups=[list(range(NCORES))],
                    ins=[cc_in.opt()],
                    outs=[cc_out.opt()],
                )
                # park the gather-result load on the (otherwise idle) ACT ring
                qmTd8 = wp.tile([128, NCORES, 8], f32)
                nc.scalar.dma_start(
                    qmTd8, cc_out[:, :].rearrange("d (p c) -> p d c", c=8)
                )

            # ---- temporal MLP -> memT [F, K] ---------------------------
            h1T = wp.tile([F // 4, K], f32)
            nc.vector.tensor_scalar_mul(h1T, tb_sb, Wt1T_sb)
            nc.vector.tensor_scalar_add(h1T, h1T, bt1T_sb)
            nc.vector.tensor_relu(h1T, h1T)
            tT_ps = pps.tile([F, K], f32, tag="small")
            nc.tensor.matmul(tT_ps, lhsT=Wt2_sb, rhs=h1T, start=True, stop=True)
            memT_sb = wp.tile([F, K], f32)
            nc.scalar.activation(memT_sb, tT_ps, AF.Identity, bias=bt2T_sb, scale=1.0)
            nc.vector.tensor_add(memT_sb, memT_sb, dgT_sb)

            # ---- gate rows g_k = mem_k @ Wg + bg  [1, H] ---------------
            def g_row(k):
                g_ps = ppb.tile([1, H], f32, tag="big")
                nc.tensor.matmul(g_ps[:, 0:512], lhsT=memT_sb[:, k : k + 1],
                                 rhs=Wg_sb[:, 0:512], start=True, stop=True)
                nc.tensor.matmul(g_ps[:, 512:1024], lhsT=memT_sb[:, k : k + 1],
                                 rhs=Wg_sb[:, 512:1024], start=True, stop=True)
                return g_ps

            g0_ps = g_row(0)
            g0_sb = wp.tile([1, H], f32, tag="g0r")
            nc.vector.tensor_add(g0_sb, g0_ps, bg_sb)
            gb0 = wp.tile([128, H], f32, tag="gb0")
            nc.gpsimd.partition_broadcast(gb0[:, :], g0_sb[:, :])
            g1_ps = g_row(1)
            g1_sb = wp.tile([1, H], f32, tag="g1r")
            nc.vector.tensor_add(g1_sb, g1_ps, bg_sb)
            gb1 = wp.tile([128, H], f32, tag="gb1")
            nc.gpsimd.partition_broadcast(gb1[:, :], g1_sb[:, :])

            # ---- matvec: fused mul+reduce on DVE -----------------------
            # rcc[p, j, k] = sum_h g_k[h] * ks[p*NT+j, h]
            rcc = wp.tile([128, NT, K], f32)
            for j in range(NT):
                for k, gb in ((0, gb0), (1, gb1)):
                    if _MATVEC_ACT:
                        prod = sp.tile([128, H], f32, tag="prod")
                        nc.vector.tensor_mul(prod, ktiles[j], gb)
                        junk = sp.tile([128, H], f32, tag="junk")
                        nc.scalar.activation(
                            junk, prod, AF.Copy,
                            accum_out=rcc[:, j, k : k + 1],
                        )
                    else:
                        prod = sp.tile([128, H], f32, tag="prod")
                        nc.vector.tensor_tensor_reduce(
                            prod, ktiles[j], gb, 1.0, 0.0,
                            ALU.mult, ALU.add, rcc[:, j, k : k + 1],
                        )

            # reshape both anchors at once to an interleaved row:
            # rTi[0, 2*s + k] = r_k[s]   (s = p*NT + j)
            rTi = wp.tile([1, K * SHARD], f32)
            nc.scalar.dma_start(rTi[:, :], rcc[:, :, :])
            # broadcast to all partitions while the scorer finishes
            rB = wp.tile([128, SHARD, K], f32)
            nc.gpsimd.partition_broadcast(rB[:, :, :], rTi[:, :])

            # ---- post-collective: qmT, scorer, weights -----------------
            # qmT[p, c] = qmean[p*8 + c]  (interleaved reshape layout)
            qmT = wp.tile([128, 8], f32)
            if use_collective:
                # sum gathered partials over d ([p, c, d] view, reduce X)
                nc.vector.tensor_reduce(
                    qmT, qmTd8[:, :, :].rearrange("p d c -> p c d"),
                    axis=mybir.AxisListType.X, op=ALU.add,
                )
            else:
                nc.scalar.dma_start(qmT, qpart_sb[:, :])
            qmTd = wp.tile([128, 8, K], f32)
            nc.vector.tensor_copy(qmTd[:, :, 0:1], qmT[:, :].rearrange("p c -> p c ()"))
            nc.vector.tensor_copy(qmTd[:, :, 1:2], qmT[:, :].rearrange("p c -> p c ()"))
            haT_ps = pps.tile([F, K], f32, tag="small")
            nc.tensor.matmul(haT_ps, lhsT=Wa1m_sb, rhs=memT_sb,
                             start=True, stop=False)
            for c in range(8):
                nc.tensor.matmul(haT_ps, lhsT=Wa1q_sb[:, c, :],
                                 rhs=qmTd[:, c, :], start=False, stop=(c == 7))
            aT_sb = wp.tile([F, K], f32)
            nc.scalar.activation(aT_sb, haT_ps, AF.Tanh, bias=ba1T_sb, scale=1.0)
            scoreT_ps = pps.tile([1, K], f32, tag="small")
            nc.tensor.matmul(scoreT_ps, lhsT=Wa2_sb, rhs=aT_sb, start=True, stop=True)
            wvT_sb = wp.tile([1, K], f32)
            nc.scalar.activation(wvT_sb, scoreT_ps, AF.Sigmoid, bias=ba2b_sb, scale=1.0)
            nc.scalar.mul(wvT_sb, wvT_sb, 1.0 / K)
            wvb = wp.tile([128, K], f32)
            nc.gpsimd.partition_broadcast(wvb[:, :], wvT_sb[:, :])

            # ---- combine anchors (128-wide), fp16 output row -----------
            o_tmp = wp.tile([128, SHARD], f32)
            nc.vector.tensor_scalar_mul(o_tmp, rB[:, :, 1], wvb[:, 1:2])
            out_sb = wp.tile([128, SHARD], f_out)
            nc.vector.scalar_tensor_tensor(
                out_sb, rB[:, :, 0], wvb[:, 0:1], o_tmp, ALU.mult, ALU.add
            )

            # ---- output: 64 x [128 rows, SHARD cols], all rows = row ---
            outv = out.rearrange("(b p) n -> b p n", p=128)
            for b in range(SEQ // 128):
                nc.sync.dma_start(outv[b], out_sb)

    nc.compile()
    return nc


def _get_prog(use_collective: bool):
    key = bool(use_collective)
    if key not in _PROG_CACHE:
        _PROG_CACHE[key] = _build(key)
    return _PROG_CACHE[key]


def _make_in_maps(inputs, use_collective: bool):
    q = np.ascontiguousarray(np.asarray(inputs["query"], np.float32)[0])  # [S,H]
    k = np.ascontiguousarray(np.asarray(inputs["key"], np.float32)[0])  # [S,H]
    common = {
        "dg": np.ascontiguousarray(np.asarray(inputs["dg_features"], np.float32)),
        "ts": np.ascontiguousarray(np.asarray(inputs["timestamps"], np.float32)),
        "Wt1": np.ascontiguousarray(np.asarray(inputs["Wt1"], np.float32)),
        "bt1": np.ascontiguousarray(np.asarray(inputs["bt1"], np.float32)),
        "Wt2": np.ascontiguousarray(np.asarray(inputs["Wt2"], np.float32)),
        "bt2": np.ascontiguousarray(np.asarray(inputs["bt2"], np.float32)),
        "Wa1": np.ascontiguousarray(np.asarray(inputs["Wa1"], np.float32)),
        "ba1": np.ascontiguousarray(np.asarray(inputs["ba1"], np.float32)),
        "Wa2": np.ascontiguousarray(np.asarray(inputs["Wa2"], np.float32)),
        "ba2": np.ascontiguousarray(np.asarray(inputs["ba2"], np.float32)),
        "Wg": np.ascontiguousarray(np.asarray(inputs["Wg"], np.float32)),
        "bg": np.ascontiguousarray(np.asarray(inputs["bg"], np.float32)),
        "scale_col": np.full((128, 1), 1.0 / 8192.0, np.float32),
    }
    in_maps = []
    for d in range(NCORES):
        m = dict(common)
        m["ks"] = np.ascontiguousarray(k[d * SHARD : (d + 1) * SHARD])
        if use_collective:
            m["qs"] = np.ascontiguousarray(q[d * SHARD : (d + 1) * SHARD])
        else:
            m["qs"] = q
        in_maps.append(m)
    return in_maps


def _run(inputs, use_collective: bool, trace: bool = False):
    from concourse.bass_utils import run_bass_kernel_spmd

    nc = _get_prog(use_collective)
    in_maps = _make_in_maps(inputs, use_collective)
    res = run_bass_kernel_spmd(
        nc, in_maps, core_ids=list(range(NCORES)), trace=trace
    )
    full = np.empty((1, 1, SEQ, SEQ), np.float32)
    for d in range(NCORES):
        full[0, 0, :, d * SHARD : (d + 1) * SHARD] = res.results[d]["out"]
    return full, res


def kernel(**inputs) -> np.ndarray:
    use_collective = os.environ.get("CA1_NO_COLLECTIVE", "0") != "1"
    if use_collective:
        for attempt in range(2):
            try:
                full, _ = _run(inputs, True)
                return full
            except Exception:
                _PROG_CACHE.pop(True, None)
        # fall back to the zero-communication variant (replicated query)
    full, _ = _run(inputs, False)
    return full
